# revision 3
# baseline (speedup 1.0000x reference)
"""Trainium2 Bass kernel for nn_DNLCLayer (grouped-conv channel attention).

Reference computation (per batch image):
  theta = gconv3x3_s2(x, theta_w) + theta_b        [RC, 56, 56]
  phi   = gconv3x3_s2(x, phi_w)   + phi_b          [RC, 56, 56]
  g     = fc(mean_hw(x)) + fc_b                    [RC]
  mat   = softmax(theta_flat @ phi_flat^T, axis=-1)  [RC, RC]
  out   = mat @ g                                  [RC, 1, 1]

Sharding: pure data parallelism — batch 32 split 4-per-core over 8 cores.
Params are tiny and replicated (pre-transformed to block-diagonal matmul
layouts on the host).

Per-core device schedule (per batch image):
  for each 128-channel tile ct (4 of them):
    DMA x[b, ct*128:+128, :] -> SBUF [128, 12544]   (the only big traffic)
    gap[:, ct] = reduce_sum over free dim (DVE)
    for each chunk of 8 output rows (7 chunks):
      9 tap matmuls accumulate into PSUM: out[40, 448]
        lhsT = block-diag weights [128, 40] (theta cols 0-7, phi cols 32-39)
        rhs  = strided SBUF view of x (stride-2 conv taps, edges clipped)
      epilogue: ACT copy + per-partition bias -> theta/phi SBUF rows 32ct..
  stage 2 (tiny): PE-transpose theta/phi chunks, gram matmul -> mat[32,32],
  softmax via (reduce_max, exp), fold g via  E^T -> [num|den], divide, DMA out.

Variants:
  f32   — conv matmuls in fp32 (4 cyc/col, exact)
  f32r  — conv matmuls with APs bitcast to float32r (1 cyc/col at N>=256)
  f16hl — x split on host into fp16 hi+lo; conv = w1*(hi+lo) + (r'*hi)/4096
          (two fp16 streams, ~fp32 accuracy at 1 cyc/col)
"""

import os
import sys
from contextlib import ExitStack

import numpy as np

sys.path.insert(0, "/opt/trn_rl_repo")

# ---- problem constants (hardcoded per contest rules) ----
FULL_B, FULL_C, FULL_H, FULL_W = 32, 512, 112, 112
CPG = 16          # channels per group
KK, STRIDE, PAD = 3, 2, 1
NCORES = 8

VARIANT = os.environ.get("DNLC_VARIANT", "f16hl")
TRACE = False          # set True from test.py to capture NTFF profile
TRACE_KW = {}
LAST_RESULT = None     # BassKernelResults of the last kernel() call
LAST_DD = None         # derived dims of the last kernel() call (for test.py)

# tap order: (1,1) first so the start=True matmul covers every PSUM element
TAPS = [(1, 1), (0, 0), (0, 1), (0, 2), (1, 0), (1, 2), (2, 0), (2, 1), (2, 2)]

R_SCALE = 4096.0  # fp16 residual weights pre-scaled by this (keeps them normal)


def _derive(BPC, C, H, W):
    assert C % 128 == 0 and H % 2 == 0 and W % 2 == 0
    d = {}
    d["BPC"], d["C"], d["H"], d["W"] = BPC, C, H, W
    d["CT"] = C // 128                 # 128-channel tiles
    d["G"] = C // CPG                  # total groups == out channels (RC)
    d["S_IN"] = H * W
    d["S_PAD"] = (H + 1) * (W + 1)     # SBUF layout with zero pad row/col
    d["OH"], d["OW"] = H // 2, W // 2
    d["S_OUT"] = d["OH"] * d["OW"]
    rpc = max(1, min(d["OH"], 448 // d["OW"]))
    while d["OH"] % rpc:
        rpc -= 1
    d["RPC"] = rpc                     # output rows per PSUM chunk
    d["NCH"] = d["OH"] // rpc          # chunks per ctile
    d["CHW"] = rpc * d["OW"]           # PSUM free size per chunk
    d["NS"] = (d["S_OUT"] + 127) // 128  # 128-wide s-chunks for stage 2
    return d


def _tap_geom(kh, kw, j, dd):
    """Output-row/col ranges for one tap in one chunk, clipping zero-padding."""
    r0 = 1 if (kh == 0 and j == 0) else 0
    nrows = dd["RPC"] - r0
    c0 = 1 if kw == 0 else 0
    ncols = dd["OW"] - c0
    oh0 = j * dd["RPC"] + r0
    ih0 = 2 * oh0 + kh - 1
    iw0 = 2 * c0 + kw - 1
    return r0, nrows, c0, ncols, ih0, iw0


# --------------------------------------------------------------------------
# host-side parameter packing
# --------------------------------------------------------------------------

def build_host_params(variant, theta_w, theta_b, phi_w, phi_b, fc_w, fc_b, dd):
    CT, G = dd["CT"], dd["G"]
    theta_w = np.asarray(theta_w, np.float32)
    phi_w = np.asarray(phi_w, np.float32)
    WM = 40 if variant in ("f32", "f32r") else 104
    wdt = np.float32 if variant in ("f32", "f32r") else np.float16
    wc = np.zeros((128, CT * 9 * WM), wdt)
    for ct in range(CT):
        for t, (kh, kw) in enumerate(TAPS):
            base = (ct * 9 + t) * WM
            for g in range(8):
                Gg = ct * 8 + g
                rows = slice(g * CPG, (g + 1) * CPG)
                th = theta_w[Gg, :, kh, kw]
                ph = phi_w[Gg, :, kh, kw]
                if variant != "f16hl":
                    wc[rows, base + g] = th
                    wc[rows, base + 32 + g] = ph
                else:
                    th1 = th.astype(np.float16)
                    ph1 = ph.astype(np.float16)
                    wc[rows, base + g] = th1
                    wc[rows, base + 32 + g] = ph1
                    wc[rows, base + 64 + g] = (
                        (th - th1.astype(np.float32)) * R_SCALE).astype(np.float16)
                    wc[rows, base + 96 + g] = (
                        (ph - ph1.astype(np.float32)) * R_SCALE).astype(np.float16)
    bcv = np.zeros((128, 2), np.float32)       # bias aligned to 32ct+g partitions
    bcv8 = np.zeros((8, 2 * CT), np.float32)   # bias aligned to partitions 0-7
    for ct in range(CT):
        for g in range(8):
            bcv[32 * ct + g, 0] = theta_b[ct * 8 + g]
            bcv[32 * ct + g, 1] = phi_b[ct * 8 + g]
            bcv8[g, 2 * ct] = theta_b[ct * 8 + g]
            bcv8[g, 2 * ct + 1] = phi_b[ct * 8 + g]
    wfc = np.zeros((128, CT * 8), np.float32)
    for ct in range(CT):
        for g in range(8):
            wfc[g * CPG:(g + 1) * CPG, ct * 8 + g] = (
                np.asarray(fc_w, np.float32)[ct * 8 + g, :, 0, 0] / dd["S_IN"])
    bfc = np.zeros((8, CT), np.float32)
    for ct in range(CT):
        bfc[:, ct] = np.asarray(fc_b, np.float32)[ct * 8:(ct + 1) * 8]
    ident = np.eye(128, dtype=np.float32)
    return {"wc": wc, "bcv": bcv, "bcv8": bcv8, "wfc": wfc, "bfc": bfc,
            "ident": ident}


# --------------------------------------------------------------------------
# device program
# --------------------------------------------------------------------------

def build_program(variant, BPC, C, H, W, nreps=1):
    import concourse.bacc as bacc
    import concourse.mybir as mybir
    import concourse.tile as tile_mod

    dd = _derive(BPC, C, H, W)
    f32 = mybir.dt.float32
    f16 = mybir.dt.float16
    f32r = mybir.dt.float32r
    Copy = mybir.ActivationFunctionType.Copy
    Ident = mybir.ActivationFunctionType.Identity
    Exp = mybir.ActivationFunctionType.Exp
    AX = mybir.AxisListType.X

    CT, G = dd["CT"], dd["G"]
    S_IN, S_OUT = dd["S_IN"], dd["S_OUT"]
    OW = dd["OW"]
    RPC, NCH, CHW, NS = dd["RPC"], dd["NCH"], dd["CHW"], dd["NS"]
    WM = 40 if variant in ("f32", "f32r") else 104

    nc = bacc.Bacc("TRN2", target_bir_lowering=False, debug=False)

    # ---- DRAM I/O ----
    S_PAD = dd["S_PAD"]
    if variant == "f16hl":
        xh_d = nc.dram_tensor("xh", [BPC * C, S_PAD], f16, kind="ExternalInput")
        xl_d = nc.dram_tensor("xl", [BPC * C, S_PAD], f16, kind="ExternalInput")
        wdt = f16
    else:
        xdt = f32r if variant == "f32r" else f32
        xs_d = nc.dram_tensor("xs", [BPC * C, S_PAD], xdt, kind="ExternalInput")
        wdt = f32r if variant == "f32r" else f32
    wc_d = nc.dram_tensor("wc", [128, CT * 9 * WM], wdt, kind="ExternalInput")
    bcv_d = nc.dram_tensor("bcv", [128, 2], f32, kind="ExternalInput")
    bcv8_d = nc.dram_tensor("bcv8", [8, 2 * CT], f32, kind="ExternalInput")
    wfc_d = nc.dram_tensor("wfc", [128, CT * 8], f32, kind="ExternalInput")
    bfc_d = nc.dram_tensor("bfc", [8, CT], f32, kind="ExternalInput")
    id_d = nc.dram_tensor("ident", [128, 128], f32, kind="ExternalInput")
    out_d = nc.dram_tensor("out_d", [BPC, G], f32, kind="ExternalOutput")

    def mmcast(ap):
        return ap

    with tile_mod.TileContext(nc) as tc, ExitStack() as es:
        cpool = es.enter_context(tc.tile_pool(name="const", bufs=1))
        xpool = es.enter_context(tc.tile_pool(name="xp", bufs=2))
        thpool = es.enter_context(tc.tile_pool(name="thp", bufs=2))
        s2pool = es.enter_context(tc.tile_pool(name="s2p", bufs=2))
        cps = es.enter_context(tc.tile_pool(name="cps", bufs=3, space="PSUM"))
        tps = es.enter_context(tc.tile_pool(name="tps", bufs=2, space="PSUM"))
        sps = es.enter_context(tc.tile_pool(name="sps", bufs=3, space="PSUM"))

        # ---- constants into SBUF ----
        wc_sb = cpool.tile([128, CT * 9 * WM], wdt, name="wc_sb", tag="wc_sb")
        nc.sync.dma_start(wc_sb[:, :], wc_d[:, :])
        bcv_sb = cpool.tile([128, 2], f32, name="bcv_sb", tag="bcv_sb")
        nc.sync.dma_start(bcv_sb[:, :], bcv_d[:, :])
        bcv8_sb = cpool.tile([8, 2 * CT], f32, name="bcv8_sb", tag="bcv8_sb")
        nc.sync.dma_start(bcv8_sb[:, :], bcv8_d[:, :])
        wfc_sb = cpool.tile([128, CT * 8], f32, name="wfc_sb", tag="wfc_sb")
        nc.sync.dma_start(wfc_sb[:, :], wfc_d[:, :])
        bfc_sb = cpool.tile([8, CT], f32, name="bfc_sb", tag="bfc_sb")
        nc.sync.dma_start(bfc_sb[:, :], bfc_d[:, :])
        id_sb = cpool.tile([128, 128], f32, name="id_sb", tag="id_sb")
        nc.sync.dma_start(id_sb[:, :], id_d[:, :])

        for b in [bb for _ in range(nreps) for bb in range(BPC)]:
            # theta/phi activations: group 8*ct+g lives at partition 32*ct+g
            th_sb = thpool.tile([128, S_OUT], f32, name="th_sb", tag="th_sb")
            ph_sb = thpool.tile([128, S_OUT], f32, name="ph_sb", tag="ph_sb")
            gap_sb = thpool.tile([128, 2 * CT], f32, name="gap_sb", tag="gap_sb")

            for ct in range(CT):
                row0 = b * C + ct * 128
                def load_padded(tname, src_d, dtyp):
                    # host sends pre-padded (H+1)x(W+1) images: fully
                    # contiguous DMA lines (~51KB/partition), no memsets
                    t = xpool.tile([128, dd["S_PAD"]], dtyp, name=tname, tag=tname)
                    for q in range(4):
                        nc.sync.dma_start(
                            t[32 * q:32 * q + 32, :],
                            src_d[row0 + 32 * q:row0 + 32 * q + 32, :])
                    return t, t.rearrange("p (h w) -> p h w", h=H + 1)

                if variant == "f16hl":
                    xht, xhv = load_padded("xht", xh_d, f16)
                    xlt, xlv = load_padded("xlt", xl_d, f16)
                    nc.vector.reduce_sum(gap_sb[:, 2 * ct:2 * ct + 1], xht[:, :],
                                         axis=AX)
                    nc.vector.reduce_sum(gap_sb[:, 2 * ct + 1:2 * ct + 2], xlt[:, :],
                                         axis=AX)
                    streams = [(xhv, True), (xlv, False)]
                else:
                    xt, xv0 = load_padded("xt", xs_d,
                                          f32r if variant == "f32r" else f32)
                    nc.vector.reduce_sum(gap_sb[:, 2 * ct:2 * ct + 1],
                                         xt[:, :].bitcast(f32) if variant == "f32r"
                                         else xt[:, :], axis=AX)
                    streams = [(xv0, True)]

                # f16hl: both streams use the same [128, 104] stationary, so
                # theta = (w1 + r'/4096) * (hi + lo) reconstructs the exact
                # fp32 weights. Adjacent (hi, lo) taps share the stationary.
                order = [(si, t) for t in range(len(TAPS))
                         for si in range(len(streams))]
                for j in range(NCH):
                    ps = cps.tile([WM, CHW], f32, name="ps", tag="ps")
                    for oi, (si, t) in enumerate(order):
                        xv, is_hi = streams[si]
                        kh, kw = TAPS[t]
                        a = 2 * RPC * j + kh
                        rhs = xv[:, a:a + 2 * RPC - 1:2,
                                 kw:kw + 2 * OW - 1:2]
                        lhs = wc_sb[:, (ct * 9 + t) * WM:(ct * 9 + t) * WM + WM]
                        nc.tensor.matmul(
                            ps[0:WM, :], mmcast(lhs), mmcast(rhs),
                            start=(oi == 0), stop=(oi == len(order) - 1))
                    csl = slice(j * CHW, (j + 1) * CHW)
                    if variant == "f16hl":
                        t1t = s2pool.tile([8, CHW], f32, name="t1t", tag="t1t")
                        t1p = s2pool.tile([8, CHW], f32, name="t1p", tag="t1p")
                        nc.scalar.activation(t1t[:, :], ps[64:72, :], Ident,
                                             bias=bcv8_sb[0:8, 2 * ct:2 * ct + 1],
                                             scale=1.0 / R_SCALE)
                        nc.scalar.activation(t1p[:, :], ps[96:104, :], Ident,
                                             bias=bcv8_sb[0:8, 2 * ct + 1:2 * ct + 2],
                                             scale=1.0 / R_SCALE)
                        nc.vector.tensor_add(th_sb[32 * ct:32 * ct + 8, csl],
                                             t1t[:, :], ps[0:8, :])
                        nc.vector.tensor_add(ph_sb[32 * ct:32 * ct + 8, csl],
                                             t1p[:, :], ps[32:40, :])
                    else:
                        nc.scalar.activation(th_sb[32 * ct:32 * ct + 8, csl],
                                             ps[0:8, :], Ident,
                                             bias=bcv_sb[32 * ct:32 * ct + 8, 0:1],
                                             scale=1.0)
                        nc.scalar.activation(ph_sb[32 * ct:32 * ct + 8, csl],
                                             ps[32:40, :], Ident,
                                             bias=bcv_sb[32 * ct:32 * ct + 8, 1:2],
                                             scale=1.0)

            # ---- stage 2 (tiny per-batch tail) ----
            if variant == "f16hl":
                gap1 = thpool.tile([128, CT], f32, name="gap1", tag="gap1")
                g3 = gap_sb.rearrange("p (c two) -> p c two", two=2)
                nc.vector.tensor_add(gap1[:, :], g3[:, :, 0], g3[:, :, 1])
            else:
                gap1 = gap_sb.rearrange("p (c two) -> p c two", two=2)[:, :, 0]

            g_ps = sps.tile([8, CT], f32, name="g_ps", tag="sp")
            for ct in range(CT):
                nc.tensor.matmul(g_ps[0:8, ct:ct + 1],
                                 wfc_sb[:, ct * 8:ct * 8 + 8],
                                 gap1[:, ct:ct + 1] if variant == "f16hl"
                                 else gap_sb[:, 2 * ct:2 * ct + 1],
                                 start=True, stop=True)
            g_sb = s2pool.tile([8, CT], f32, name="g_sb", tag="g_sb")
            nc.vector.tensor_add(g_sb[:, :], g_ps[0:8, :], bfc_sb[0:8, :])
            gv = s2pool.tile([G, 2], f32, name="gv", tag="gv")
            nc.vector.memset(gv[:, 1:2], 1.0)
            for ct in range(CT):
                nc.sync.dma_start(gv[8 * ct:8 * ct + 8, 0:1],
                                  g_sb[0:8, ct:ct + 1])

            mat_ps = sps.tile([G, G], f32, name="mat_ps", tag="sp")
            for m in range(NS):
                L = min(128, S_OUT - m * 128)
                ssl = slice(m * 128, m * 128 + L)
                tp_ps = tps.tile([128, 256], f32, name="tp_ps", tag="tp_ps")
                nc.tensor.transpose(tp_ps[0:L, 0:128], th_sb[:, ssl],
                                    id_sb[:, :])
                nc.tensor.transpose(tp_ps[0:L, 128:256], ph_sb[:, ssl],
                                    id_sb[:, :])
                # compact the 32-strided valid columns while copying to SBUF
                # (walrus: matmul stationary APs may have only one free dim)
                tp_sb = s2pool.tile([128, 2 * G], f32, name="tp_sb", tag="tp_sb")
                ppv = tp_ps.rearrange("p (h c k) -> p h c k", h=2, c=CT)
                sbv = tp_sb.rearrange("p (h c k) -> p h c k", h=2, c=CT, k=8)
                nc.scalar.activation(sbv[0:L, :, :, :], ppv[0:L, :, :, 0:8], Copy)
                nc.tensor.matmul(mat_ps[0:G, 0:G],
                                 tp_sb[0:L, 0:G], tp_sb[0:L, G:2 * G],
                                 start=(m == 0), stop=(m == NS - 1))

            nmax = s2pool.tile([G, 1], f32, name="nmax", tag="nmax")
            nc.vector.reduce_max(nmax[:, :], mat_ps[0:G, :], axis=AX, negate=True)
            e_sb = s2pool.tile([G, G], f32, name="e_sb", tag="e_sb")
            nc.scalar.activation(e_sb[:, :], mat_ps[0:G, :], Exp,
                                 bias=nmax[:, 0:1], scale=1.0)
            et_ps = sps.tile([G, G], f32, name="et_ps", tag="sp")
            nc.tensor.transpose(et_ps[0:G, 0:G], e_sb[:, :], id_sb[0:G, 0:G])
            et_sb = s2pool.tile([G, G], f32, name="et_sb", tag="et_sb")
            nc.scalar.activation(et_sb[:, :], et_ps[0:G, :], Copy)
            nd_ps = sps.tile([G, 2], f32, name="nd_ps", tag="sp")
            nc.tensor.matmul(nd_ps[0:G, 0:2], et_sb[:, :], gv[:, :],
                             start=True, stop=True)
            rec = s2pool.tile([G, 1], f32, name="rec", tag="rec")
            nc.vector.reciprocal(rec[:, :], nd_ps[0:G, 1:2])
            res = s2pool.tile([G, 1], f32, name="res", tag="res")
            nc.vector.tensor_mul(res[:, :], nd_ps[0:G, 0:1], rec[:, :])
            nc.sync.dma_start(out_d[b:b + 1, :], res[0:G, 0:1])

    nc.compile()
    return nc


_PROGRAM_CACHE = {}


def _get_program(variant, BPC, C, H, W, nreps=1):
    key = (variant, BPC, C, H, W, nreps)
    if key not in _PROGRAM_CACHE:
        _PROGRAM_CACHE[key] = build_program(variant, BPC, C, H, W, nreps)
    return _PROGRAM_CACHE[key]


def make_in_maps(variant, x, params, dd, ncores):
    """x: [B, C, H, W] float32. Returns one input dict per core."""
    B, C, H, W = x.shape
    BPC = B // ncores
    S_PAD = dd["S_PAD"]
    xf = x.reshape(ncores, BPC * C, H, W)
    common = {"wc": params["wc"], "bcv": params["bcv"], "bcv8": params["bcv8"],
              "wfc": params["wfc"], "bfc": params["bfc"], "ident": params["ident"]}
    in_maps = []
    for c in range(ncores):
        m = dict(common)
        if variant == "f16hl":
            hi = xf[c].astype(np.float16)
            lo = (xf[c] - hi.astype(np.float32)).astype(np.float16)
            hp = np.zeros((BPC * C, H + 1, W + 1), np.float16)
            hp[:, 1:, 1:] = hi
            lp = np.zeros((BPC * C, H + 1, W + 1), np.float16)
            lp[:, 1:, 1:] = lo
            m["xh"] = hp.reshape(BPC * C, S_PAD)
            m["xl"] = lp.reshape(BPC * C, S_PAD)
        else:
            xp = np.zeros((BPC * C, H + 1, W + 1), np.float32)
            xp[:, 1:, 1:] = xf[c]
            m["xs"] = xp.reshape(BPC * C, S_PAD)
        in_maps.append(m)
    return in_maps


def kernel(x, theta_w, theta_b, phi_w, phi_b, fc_w, fc_b):
    global LAST_RESULT, LAST_DD
    from concourse import bass_utils

    x = np.asarray(x, np.float32)
    B, C, H, W = x.shape
    BPC = B // NCORES
    dd = _derive(BPC, C, H, W)
    LAST_DD = dd
    params = build_host_params(VARIANT, np.asarray(theta_w), np.asarray(theta_b),
                               np.asarray(phi_w), np.asarray(phi_b),
                               np.asarray(fc_w), np.asarray(fc_b), dd)
    nc = _get_program(VARIANT, BPC, C, H, W)
    in_maps = make_in_maps(VARIANT, x, params, dd, NCORES)
    res = bass_utils.run_bass_kernel_spmd(
        nc, in_maps, core_ids=list(range(NCORES)), trace=TRACE, **TRACE_KW)
    LAST_RESULT = res
    out = np.stack([res.results[c]["out_d"] for c in range(NCORES)], axis=0)
    return out.reshape(B, dd["G"])[:, :, None, None].astype(np.float32)

